# revision 15
# baseline (speedup 1.0000x reference)
"""HR2O_NL sparse-attention kernel for 8 Trainium2 NeuronCores.

Sharding: data-parallel over ROI groups (videos packed onto 8 cores so whole
groups stay local; an exact 64-per-core partition is searched first, LPT
fallback). Weights + GN params replicated.

Layout: all activations are (p,i)-major on device: flat index p*CAP + i with
p = y*7+x the spatial position and i the ROI slot. 3x3 convs run as 9
tap-trimmed matmuls per output row (row-bank structure): both the rhs stream
and the PSUM out AP then have contiguous runs of CAP, which the PE streams at
~1 col/cycle (no zero-pad FLOPs, no transposes anywhere). Attention, GroupNorm
and the residual all stay in (p,i)-major.
"""
import sys, types
import numpy as np
import ml_dtypes

import concourse.bass as bass
import concourse.mybir as mybir
import concourse.tile as tile
from concourse.bass_utils import run_bass_kernel_spmd

BF = mybir.dt.bfloat16
F32 = mybir.dt.float32
C = 512
NCORE = 8
P = 49
AF = mybir.ActivationFunctionType


def _install_profhook():
    if 'antenv.axon_hooks' in sys.modules:
        return
    try:
        from trn_agent_boot.trn_boot import _ntff_profile_via_ctypes
        hook = _ntff_profile_via_ctypes('/opt/axon/libaxon_pjrt.so')
    except Exception:
        hook = None
    m = types.ModuleType('antenv.axon_hooks')
    m.get_axon_ntff_profile_hook = lambda: hook
    sys.modules['antenv.axon_hooks'] = m


def _walk_blocks(bb):
    yield bb
    for inner in getattr(bb, 'blocks', []) or []:
        yield from _walk_blocks(inner)


def _split_multiwait(nc):
    # this walrus build accepts one sync wait per instruction
    fn = nc.m.functions[0]
    for bb in list(_walk_blocks(fn)):
        insts = getattr(bb, 'instructions', None)
        if not insts:
            continue
        new_list, changed = [], False
        for inst in insts:
            si = inst.sync_info
            if si is not None and si.on_wait is not None and len(si.on_wait) > 1:
                waits = list(si.on_wait)
                for j, w in enumerate(waits[:-1]):
                    d = mybir.InstDrain(name=f"{inst.name}_ws{j}", ins=[], outs=[])
                    d.engine = inst.engine
                    d.sync_info = mybir.SyncInfo(on_wait=[w], on_update=[])
                    new_list.append(d)
                si.on_wait = [waits[-1]]
                changed = True
            new_list.append(inst)
        if changed:
            insts[:] = new_list


# taps: (dy, dx) with (0,0) first so the start=True matmul covers the full bank
TAPS = [(0, 0)] + [(dy, dx) for dy in (-1, 0, 1) for dx in (-1, 0, 1)
                   if not (dy == 0 and dx == 0)]

_NC_CACHE = {}


def _build(CAP):
    if CAP in _NC_CACHE:
        return _NC_CACHE[CAP]
    NPOS = CAP * P
    W7 = 7 * CAP          # one output row: 7 x-positions for all rois
    nc = bass.Bass("TRN2", target_bir_lowering=False, debug=False,
                   num_devices=NCORE)
    xp_d = nc.dram_tensor("xp", [4, 128, NPOS], BF, kind="ExternalInput")
    xint_d = nc.dram_tensor("xint", [4, 128, NPOS], F32, kind="ExternalInput")
    wq_d = nc.dram_tensor("wq", [4, 128, 9, 4, 128], BF, kind="ExternalInput")
    wk_d = nc.dram_tensor("wk", [4, 128, 9, 4, 128], BF, kind="ExternalInput")
    wv_d = nc.dram_tensor("wv", [4, 128, 9, 4, 128], BF, kind="ExternalInput")
    wo_d = nc.dram_tensor("wo", [4, 128, 9, 4, 128], BF, kind="ExternalInput")
    mask_d = nc.dram_tensor("mask", [CAP, CAP], F32, kind="ExternalInput")
    y_d = nc.dram_tensor("y", [4, 128, NPOS], F32, kind="ExternalOutput")
    v_dram = nc.dram_tensor("v_sc", [CAP, 4, P, 128], BF)

    def conv_row_mms(b):
        """matmul descriptors for output row b: (ci, tap_idx, src_off, dst_off, rx)"""
        out = []
        for ci in range(4):
            for (dy, dx) in TAPS:
                yy = b + dy
                if not (0 <= yy < 7):
                    continue
                rx = 7 - abs(dx)
                src_off = (yy * 7 + max(dx, 0)) * CAP
                dst_off = max(-dx, 0) * CAP
                out.append((ci, (dy + 1) * 3 + (dx + 1), src_off, dst_off, rx))
        return out

    def wslice_ap(wd, ci, cto):
        return bass.AP(tensor=wd[:].tensor, offset=ci * 589824 + cto * 128,
                       ap=[[4608, 128], [512, 9], [1, 128]])

    with tile.TileContext(nc) as tc:
        with (
            tc.tile_pool(name="persist", bufs=1) as pp,
            tc.tile_pool(name="wts", bufs=2) as wts,
            tc.tile_pool(name="drains", bufs=4) as drp,
            tc.tile_pool(name="pconv", bufs=3, space="PSUM") as pconv,
            tc.tile_pool(name="pqk", bufs=2, space="PSUM") as pqk,
            tc.tile_pool(name="pav", bufs=2, space="PSUM") as pav,
            tc.tile_pool(name="pst", bufs=1, space="PSUM") as pst,
        ):
            xt = [pp.tile([128, NPOS], BF, name=f"xp{c}") for c in range(4)]
            for c in range(4):
                nc.sync.dma_start(out=xt[c][:], in_=xp_d[c])
            mask_t = pp.tile([CAP, CAP], F32, name="mask")
            nc.sync.dma_start(out=mask_t[:], in_=mask_d[:])
            mask7 = pp.tile([CAP, W7], F32, name="mask7")
            for r in range(7):
                nc.vector.tensor_copy(mask7[:, r * CAP:(r + 1) * CAP], mask_t[:])
            attw = pp.tile([CAP, NPOS], BF, name="attw")
            virt = [pp.tile([128, NPOS], BF, name=f"virt{t}") for t in range(4)]
            rows_cm = tc.tile_pool(name="rows", bufs=2)
            rows = rows_cm.__enter__()
            ones64 = pp.tile([CAP, 1], BF, name="ones64")
            nc.vector.memset(ones64[:], 1.0)
            ones1b = pp.tile([1, 128], BF, name="ones1b")
            nc.vector.memset(ones1b[:], 1.0)
            onesf = pp.tile([128, 1], BF, name="onesf")
            nc.vector.memset(onesf[:], 1.0)

            vthp_cm = tc.tile_pool(name="vthp", bufs=2)
            vthp = vthp_cm.__enter__()
            qkp_cm = tc.tile_pool(name="qk", bufs=1)
            qkp = qkp_cm.__enter__()
            q_s = [qkp.tile([128, NPOS], BF, name=f"q{t}") for t in range(4)]
            k_s = [qkp.tile([128, NPOS], BF, name=f"k{t}") for t in range(4)]

            # ---------------- phase 1: q,k convs ----------------
            ndrain = 0
            for wd, dst in ((wq_d, q_s), (wk_d, k_s)):
                for cto in range(4):
                    wt = wts.tile([128, 4, 9, 128], BF, name="wt", tag="wt")
                    for ci in range(4):
                        nc.sync.dma_start(out=wt[:, ci, :, :], in_=wslice_ap(wd, ci, cto))
                    for b in range(7):
                        acc = pconv.tile([128, W7], F32, name="acc", tag="acc")
                        mms = conv_row_mms(b)
                        for mi, (ci, ti, so, do, rx) in enumerate(mms):
                            rhs = bass.AP(tensor=xt[ci].tensor,
                                          offset=xt[ci].offset + so,
                                          ap=[xt[ci].ap[0], [CAP, rx], [1, CAP]])
                            outap = bass.AP(tensor=acc.tensor, offset=acc.offset + do,
                                            ap=[acc.ap[0], [CAP, rx], [1, CAP]])
                            nc.tensor.matmul(outap, wt[:, ci, ti, :], rhs,
                                             start=(mi == 0), stop=(mi == len(mms) - 1),
                                             skip_group_check=True)
                        dslice = dst[cto][:, b * W7:(b + 1) * W7]
                        if ndrain % 2 == 0:
                            nc.vector.tensor_copy(dslice, acc[:])
                        else:
                            nc.scalar.activation(dslice, acc[:], func=AF.Copy)
                        ndrain += 1

            # ---------------- phase 2a: QK^T + mask + exp + rowsum ----------------
            rsum = rows.tile([1, NPOS], F32, name="rsum", tag="row")
            rec_bf = pp.tile([1, NPOS], BF, name="recbf")
            for pg in range(7):
                aps = pqk.tile([CAP, W7], F32, name="aps", tag="aps")
                for pp_i in range(7):
                    p = pg * 7 + pp_i
                    for ct in range(4):
                        lhsT = k_s[ct][:, p * CAP:(p + 1) * CAP]
                        rhs = q_s[ct][:, p * CAP:(p + 1) * CAP]
                        nc.tensor.matmul(aps[:, pp_i * CAP:(pp_i + 1) * CAP],
                                         lhsT, rhs, start=(ct == 0), stop=(ct == 3))
                attf = drp.tile([CAP, W7], F32, name="attf", tag="attf")
                nc.vector.tensor_add(attf[:], aps[:], mask7[:])
                nc.scalar.activation(attw[:, pg * W7:(pg + 1) * W7], attf[:],
                                     func=AF.Exp)
                rs_ps = pst.tile([1, W7], F32, name="rsps", tag="rsps")
                nc.tensor.matmul(rs_ps[:], ones64[:],
                                 attw[:, pg * W7:(pg + 1) * W7],
                                 start=True, stop=True)
                nc.vector.tensor_copy(rsum[:, pg * W7:(pg + 1) * W7], rs_ps[:])
            nc.vector.reciprocal(rsum[:], rsum[:])
            nc.vector.tensor_copy(rec_bf[:], rsum[:])
            qkp_cm.__exit__(None, None, None)
            rpp_cm = tc.tile_pool(name="rpp", bufs=1)
            rpp = rpp_cm.__enter__()
            rp = [rpp.tile([128, NPOS], BF, name=f"rp{t}") for t in range(4)]

            # ---------------- v conv (cto 0,1) ----------------
            def v_conv_cto(cto):
                wt = wts.tile([128, 4, 9, 128], BF, name="wtv", tag="wt")
                for ci in range(4):
                    nc.sync.dma_start(out=wt[:, ci, :, :], in_=wslice_ap(wv_d, ci, cto))
                for b in range(7):
                    acc = pconv.tile([128, W7], F32, name="accv", tag="acc")
                    mms = conv_row_mms(b)
                    for mi, (ci, ti, so, do, rx) in enumerate(mms):
                        rhs = bass.AP(tensor=xt[ci].tensor, offset=xt[ci].offset + so,
                                      ap=[xt[ci].ap[0], [CAP, rx], [1, CAP]])
                        outap = bass.AP(tensor=acc.tensor, offset=acc.offset + do,
                                        ap=[acc.ap[0], [CAP, rx], [1, CAP]])
                        nc.tensor.matmul(outap, wt[:, ci, ti, :], rhs,
                                         start=(mi == 0), stop=(mi == len(mms) - 1),
                                         skip_group_check=True)
                    vs = drp.tile([128, W7], BF, name="vs", tag="vs")
                    nc.vector.tensor_copy(vs[:], acc[:])
                    # store to v_dram[i, cto, p, c], one DMA per spatial p
                    for pp_i in range(7):
                        p = b * 7 + pp_i
                        dstap = bass.AP(tensor=v_dram[:].tensor,
                                        offset=cto * P * 128 + p * 128,
                                        ap=[[1, 128], [4 * P * 128, CAP]])
                        nc.sync.dma_start(
                            out=dstap, in_=vs[:, pp_i * CAP:(pp_i + 1) * CAP])

            v_conv_cto(0)
            v_conv_cto(1)

            # recip bcast + attw scale (tensor MMs land after v cto1 in engine order)
            for pg in range(7):
                rb = pav.tile([CAP, W7], F32, name="rb", tag="rb")
                rhs = bass.AP(tensor=rec_bf.tensor, offset=rec_bf.offset + pg * W7,
                              ap=[rec_bf.ap[0], [1, W7]])
                o1 = bass.AP(tensor=ones1b.tensor, offset=ones1b.offset,
                             ap=[ones1b.ap[0], [1, CAP]])
                nc.tensor.matmul(rb[:], o1, rhs, start=True, stop=True)
                aw = attw[:, pg * W7:(pg + 1) * W7]
                nc.vector.tensor_mul(aw, aw, rb[:])

            v_conv_cto(2)

            # ---------------- AV ----------------
            def av_ct(ct):
                vth = vthp.tile([CAP, P, 128], BF, name=f"vth{ct}", tag="vth")
                src = bass.AP(tensor=v_dram[:].tensor, offset=ct * P * 128,
                              ap=[[4 * P * 128, CAP], [1, P * 128]])
                nc.sync.dma_start(out=vth[:], in_=src)
                for pg in range(7):
                    av = pav.tile([128, W7], F32, name="av", tag="rb")
                    for pp_i in range(7):
                        p = pg * 7 + pp_i
                        nc.tensor.matmul(av[:, pp_i * CAP:(pp_i + 1) * CAP],
                                         vth[:, p, :],
                                         attw[:, p * CAP:(p + 1) * CAP],
                                         start=True, stop=True)
                    dslice = virt[ct][:, pg * W7:(pg + 1) * W7]
                    if (ct + pg) % 2 == 0:
                        nc.vector.tensor_copy(dslice, av[:])
                    else:
                        nc.scalar.activation(dslice, av[:], func=AF.Copy)

            av_ct(0)
            av_ct(1)
            v_conv_cto(3)
            av_ct(2)
            av_ct(3)

            # ---------------- GN stats ----------------
            s1 = rows.tile([1, NPOS], F32, name="s1", tag="row")
            s2 = rows.tile([1, NPOS], F32, name="s2", tag="row")
            for pg in range(7):
                sqs = []
                for ct in range(4):
                    sq = drp.tile([128, W7], BF, name="sq", tag="attf")
                    nc.scalar.activation(sq[:], virt[ct][:, pg * W7:(pg + 1) * W7],
                                         func=AF.Square)
                    sqs.append(sq)
                s1_ps = pst.tile([1, W7], F32, name="s1ps", tag="rsps")
                s2_ps = pst.tile([1, W7], F32, name="s2ps", tag="rsps")
                for ct in range(4):
                    nc.tensor.matmul(s1_ps[:], onesf[:],
                                     virt[ct][:, pg * W7:(pg + 1) * W7],
                                     start=(ct == 0), stop=(ct == 3))
                for ct in range(4):
                    nc.tensor.matmul(s2_ps[:], onesf[:], sqs[ct][:],
                                     start=(ct == 0), stop=(ct == 3))
                nc.vector.tensor_copy(s1[:, pg * W7:(pg + 1) * W7], s1_ps[:])
                nc.vector.tensor_copy(s2[:, pg * W7:(pg + 1) * W7], s2_ps[:])

            s1i = pp.tile([1, CAP], F32, name="s1i")
            s2i = pp.tile([1, CAP], F32, name="s2i")
            for src_t, dsti in ((s1, s1i), (s2, s2i)):
                v3 = bass.AP(tensor=src_t.tensor, offset=src_t.offset,
                             ap=[src_t.ap[0], [1, CAP], [CAP, P]])
                nc.vector.reduce_sum(dsti[:], v3, axis=mybir.AxisListType.X)
            inv_n = 1.0 / (C * P)
            mean_r = pp.tile([1, CAP], F32, name="meanr")
            var_r = pp.tile([1, CAP], F32, name="varr")
            msq = pp.tile([1, CAP], F32, name="msq")
            eps_t = pp.tile([1, 1], F32, name="eps")
            nc.vector.memset(eps_t[:], 1e-5)
            nc.vector.tensor_scalar_mul(mean_r[:], s1i[:], inv_n)
            nc.vector.tensor_scalar_mul(var_r[:], s2i[:], inv_n)
            nc.vector.tensor_mul(msq[:], mean_r[:], mean_r[:])
            nc.vector.tensor_sub(var_r[:], var_r[:], msq[:])
            rstd_bf = pp.tile([1, CAP], BF, name="rstdbf")
            negb_bf = pp.tile([1, CAP], BF, name="negbbf")
            nc.scalar.activation(var_r[:], var_r[:], func=AF.Sqrt,
                                 bias=eps_t[:], scale=1.0)
            nc.vector.reciprocal(var_r[:], var_r[:])   # var_r := rstd
            nc.vector.tensor_mul(msq[:], mean_r[:], var_r[:])
            nc.vector.tensor_scalar_mul(msq[:], msq[:], -1.0)  # msq := negb
            nc.vector.tensor_copy(rstd_bf[:], var_r[:])
            nc.vector.tensor_copy(negb_bf[:], msq[:])

            # ---------------- normalize + relu (per pg, pipelined) ----------------
            for pg in range(7):
                rstd_ps = pav.tile([128, W7], F32, name="rstdps", tag="rb")
                negb_ps = pst.tile([128, W7], F32, name="negbps", tag="rsps")
                rr = bass.AP(tensor=rstd_bf.tensor, offset=rstd_bf.offset,
                             ap=[rstd_bf.ap[0], [0, 7], [1, CAP]])
                nb = bass.AP(tensor=negb_bf.tensor, offset=negb_bf.offset,
                             ap=[negb_bf.ap[0], [0, 7], [1, CAP]])
                nc.tensor.matmul(rstd_ps[:], ones1b[:], rr, start=True, stop=True)
                nc.tensor.matmul(negb_ps[:], ones1b[:], nb, start=True, stop=True)
                for ct in range(4):
                    t = drp.tile([128, W7], BF, name="tno", tag="attf")
                    vslice = virt[ct][:, pg * W7:(pg + 1) * W7]
                    nc.vector.tensor_mul(t[:], vslice, rstd_ps[:])
                    nc.vector.tensor_add(t[:], t[:], negb_ps[:])
                    nc.scalar.activation(rp[ct][:, pg * W7:(pg + 1) * W7], t[:],
                                         func=AF.Relu)

            # ---------------- phase 3: out conv + residual ----------------
            for cto in range(4):
                wt = wts.tile([128, 4, 9, 128], BF, name="wt3", tag="wt")
                for ci in range(4):
                    nc.sync.dma_start(out=wt[:, ci, :, :], in_=wslice_ap(wo_d, ci, cto))
                for b in range(7):
                    xit = drp.tile([128, W7], F32, name="xit", tag="xit")
                    nc.sync.dma_start(
                        out=xit[:], in_=xint_d[cto][:, b * W7:(b + 1) * W7])
                    acc = pconv.tile([128, W7], F32, name="acc3", tag="acc")
                    mms = conv_row_mms(b)
                    for mi, (ci, ti, so, do, rx) in enumerate(mms):
                        rhs = bass.AP(tensor=rp[ci].tensor, offset=rp[ci].offset + so,
                                      ap=[rp[ci].ap[0], [CAP, rx], [1, CAP]])
                        outap = bass.AP(tensor=acc.tensor, offset=acc.offset + do,
                                        ap=[acc.ap[0], [CAP, rx], [1, CAP]])
                        nc.tensor.matmul(outap, wt[:, ci, ti, :], rhs,
                                         start=(mi == 0), stop=(mi == len(mms) - 1),
                                         skip_group_check=True)
                    o = drp.tile([128, W7], F32, name="o", tag="o")
                    nc.vector.tensor_add(o[:], acc[:], xit[:])
                    nc.sync.dma_start(out=y_d[cto][:, b * W7:(b + 1) * W7], in_=o[:])

            rpp_cm.__exit__(None, None, None)
            vthp_cm.__exit__(None, None, None)
            rows_cm.__exit__(None, None, None)

    _split_multiwait(nc)
    _NC_CACHE[CAP] = nc
    return nc


def _shard(rois):
    """Pack videos onto 8 cores: exact-64 partition if possible, else LPT."""
    vid = rois[:, 0].astype(np.int64)
    sizes = np.bincount(vid, minlength=vid.max() + 1)
    nvid = len(sizes)
    total = int(sizes.sum())
    target = total // NCORE
    v2c = None
    if total % NCORE == 0:
        import random
        rng = random.Random(0)
        for _ in range(300):
            remaining = set(range(nvid))
            bins = []
            ok = True
            for _b in range(NCORE):
                items = [v for v in remaining]
                rng.shuffle(items)
                reach = {0: []}
                for v in items:
                    s = int(sizes[v])
                    if s == 0:
                        continue
                    new = {}
                    for tot, sub in reach.items():
                        t2 = tot + s
                        if t2 <= target and t2 not in reach and t2 not in new:
                            new[t2] = sub + [v]
                    reach.update(new)
                    if target in reach:
                        break
                if target not in reach:
                    ok = False
                    break
                sub = reach[target]
                bins.append(sub)
                remaining -= set(sub)
            if ok and not any(sizes[v] > 0 for v in remaining):
                v2c = np.zeros(nvid, np.int64)
                for c, b in enumerate(bins):
                    for v in b:
                        v2c[v] = c
                break
    if v2c is None:
        order = np.argsort(-sizes, kind='stable')
        loads = np.zeros(NCORE, np.int64)
        v2c = np.zeros(nvid, np.int64)
        for v in order:
            c = int(np.argmin(loads))
            loads[c] += sizes[v]
            v2c[v] = c
    core_of_roi = v2c[vid]
    idxs = [np.nonzero(core_of_roi == c)[0] for c in range(NCORE)]
    cap = max(64, max(len(ix) for ix in idxs))
    assert 7 * cap * 4 <= 2048 * 1, f"row bank overflow: cap={cap}"
    return idxs, vid, cap


def kernel(x, rois, w_q, w_k, w_v, w_out, gamma, beta):
    _install_profhook()
    x = np.asarray(x, np.float32)
    rois = np.asarray(rois)
    assert np.allclose(np.asarray(gamma), 1.0) and np.allclose(np.asarray(beta), 0.0), \
        "kernel folds GN affine assuming gamma=1, beta=0"
    idxs, vid, CAP = _shard(rois)
    nc = _build(CAP)
    NPOS = CAP * P

    def wprep(w, scale=1.0):
        # [co, ci, 1, 3, 3] -> [ci(4,128), tap(ky*3+kx), co(4,128)] bf16
        a = (np.asarray(w, np.float32)[:, :, 0] * scale).transpose(1, 2, 3, 0)
        return np.ascontiguousarray(
            a.reshape(4, 128, 9, 4, 128)).astype(ml_dtypes.bfloat16)

    wq = wprep(w_q, 1.0 / np.sqrt(np.float32(C)))
    wk, wv, wo = wprep(w_k), wprep(w_v), wprep(w_out)

    in_maps = []
    for c in range(NCORE):
        ix = idxs[c]
        n = len(ix)
        # (p,i)-major: [C, 49, CAP]
        xq = np.zeros((C, P, CAP), np.float32)
        xq[:, :, :n] = x[ix, :, 0].reshape(n, C, P).transpose(1, 2, 0)
        xp = np.ascontiguousarray(xq.reshape(4, 128, NPOS)).astype(ml_dtypes.bfloat16)
        xint = np.ascontiguousarray(xq.reshape(4, 128, NPOS))
        ids = np.full(CAP, -1, np.int64)
        ids[:n] = vid[ix]
        ids[n:] = 10 ** 6 + np.arange(CAP - n)
        mask = np.where(ids[:, None] == ids[None, :], 0.0, -1e30).astype(np.float32)
        in_maps.append(dict(xp=xp, xint=xint, wq=wq, wk=wk, wv=wv, wo=wo, mask=mask))

    res = run_bass_kernel_spmd(nc, in_maps, list(range(NCORE)))
    kernel.last_exec_ns = res.exec_time_ns

    out = np.empty((512, C, 1, 7, 7), np.float32)
    for c in range(NCORE):
        ix = idxs[c]
        n = len(ix)
        yc = res.results[c]["y"].reshape(C, P, CAP).transpose(2, 0, 1)
        out[ix] = yc[:n].reshape(n, C, 1, 7, 7)
    return out


# revision 22
# speedup vs baseline: 9.2184x; 9.2184x over previous
"""HR2O_NL sparse-attention kernel for 8 Trainium2 NeuronCores.

Sharding: data-parallel over ROI groups (videos packed onto 8 cores so whole
groups stay local; an exact 64-per-core partition is searched first, LPT
fallback). Weights + GN params replicated.

Layout: all activations are (p,i)-major on device: flat index p*CAP + i with
p = y*7+x the spatial position and i the ROI slot. 3x3 convs run as 9
tap-trimmed matmuls per output row (row-bank structure): both the rhs stream
and the PSUM out AP then have contiguous runs of CAP, which the PE streams at
~1 col/cycle (no zero-pad FLOPs, no transposes anywhere). Attention, GroupNorm
and the residual all stay in (p,i)-major.
"""
import sys, types
import numpy as np
import ml_dtypes

import concourse.bass as bass
import concourse.mybir as mybir
import concourse.tile as tile
from concourse.bass_utils import run_bass_kernel_spmd

BF = mybir.dt.bfloat16
F32 = mybir.dt.float32
C = 512
NCORE = 8
P = 49
AF = mybir.ActivationFunctionType


def _install_profhook():
    if 'antenv.axon_hooks' in sys.modules:
        return
    try:
        from trn_agent_boot.trn_boot import _ntff_profile_via_ctypes
        hook = _ntff_profile_via_ctypes('/opt/axon/libaxon_pjrt.so')
    except Exception:
        hook = None
    m = types.ModuleType('antenv.axon_hooks')
    m.get_axon_ntff_profile_hook = lambda: hook
    sys.modules['antenv.axon_hooks'] = m


def _walk_blocks(bb):
    yield bb
    for inner in getattr(bb, 'blocks', []) or []:
        yield from _walk_blocks(inner)


def _split_multiwait(nc):
    # this walrus build accepts one sync wait per instruction
    fn = nc.m.functions[0]
    for bb in list(_walk_blocks(fn)):
        insts = getattr(bb, 'instructions', None)
        if not insts:
            continue
        new_list, changed = [], False
        for inst in insts:
            si = inst.sync_info
            if si is not None and si.on_wait is not None and len(si.on_wait) > 1:
                waits = list(si.on_wait)
                for j, w in enumerate(waits[:-1]):
                    d = mybir.InstDrain(name=f"{inst.name}_ws{j}", ins=[], outs=[])
                    d.engine = inst.engine
                    d.sync_info = mybir.SyncInfo(on_wait=[w], on_update=[])
                    new_list.append(d)
                si.on_wait = [waits[-1]]
                changed = True
            new_list.append(inst)
        if changed:
            insts[:] = new_list


# taps: (dy, dx) with (0,0) first so the start=True matmul covers the full bank
TAPS = [(0, 0)] + [(dy, dx) for dy in (-1, 0, 1) for dx in (-1, 0, 1)
                   if not (dy == 0 and dx == 0)]

_NC_CACHE = {}


def _build(CAP):
    if CAP in _NC_CACHE:
        return _NC_CACHE[CAP]
    NPOS = CAP * P
    W7 = 7 * CAP          # one output row: 7 x-positions for all rois
    nc = bass.Bass("TRN2", target_bir_lowering=False, debug=False,
                   num_devices=NCORE)
    xp_d = nc.dram_tensor("xp", [4, 128, NPOS], BF, kind="ExternalInput")
    xint_d = nc.dram_tensor("xint", [4, 128, NPOS], F32, kind="ExternalInput")
    wq_d = nc.dram_tensor("wq", [4, 128, 9, 4, 128], BF, kind="ExternalInput")
    wk_d = nc.dram_tensor("wk", [4, 128, 9, 4, 128], BF, kind="ExternalInput")
    wv_d = nc.dram_tensor("wv", [4, 128, 9, 4, 128], BF, kind="ExternalInput")
    wo_d = nc.dram_tensor("wo", [4, 128, 9, 4, 128], BF, kind="ExternalInput")
    mask_d = nc.dram_tensor("mask", [CAP, CAP], F32, kind="ExternalInput")
    y_d = nc.dram_tensor("y", [4, 128, NPOS], F32, kind="ExternalOutput")
    v_dram = nc.dram_tensor("v_sc", [CAP, 4, 128, P], BF)   # [i, ct, c, p]

    def conv_row_mms(b):
        """matmul descriptors for output row b: (ci, tap_idx, src_off, dst_off, rx)"""
        out = []
        for ci in range(4):
            for (dy, dx) in TAPS:
                yy = b + dy
                if not (0 <= yy < 7):
                    continue
                rx = 7 - abs(dx)
                src_off = (yy * 7 + max(dx, 0)) * CAP
                dst_off = max(-dx, 0) * CAP
                out.append((ci, (dy + 1) * 3 + (dx + 1), src_off, dst_off, rx))
        return out

    def wslice_ap(wd, ci, cto):
        return bass.AP(tensor=wd[:].tensor, offset=ci * 589824 + cto * 128,
                       ap=[[4608, 128], [512, 9], [1, 128]])

    with tile.TileContext(nc) as tc:
        with (
            tc.tile_pool(name="persist", bufs=1) as pp,
            tc.tile_pool(name="wts", bufs=2) as wts,
            tc.tile_pool(name="drains", bufs=4) as drp,
            tc.tile_pool(name="pconv", bufs=3, space="PSUM") as pconv,
            tc.tile_pool(name="pqk", bufs=2, space="PSUM") as pqk,
            tc.tile_pool(name="pav", bufs=2, space="PSUM") as pav,
            tc.tile_pool(name="pst", bufs=1, space="PSUM") as pst,
        ):
            xt = [pp.tile([128, NPOS], BF, name=f"xp{c}") for c in range(4)]
            for c in range(4):
                nc.sync.dma_start(out=xt[c][:], in_=xp_d[c])
            mask_t = pp.tile([CAP, CAP], F32, name="mask")
            nc.sync.dma_start(out=mask_t[:], in_=mask_d[:])
            mask7 = pp.tile([CAP, W7], F32, name="mask7")
            for r in range(7):
                nc.vector.tensor_copy(mask7[:, r * CAP:(r + 1) * CAP], mask_t[:])
            attw = pp.tile([CAP, NPOS], BF, name="attw")
            virt = [pp.tile([128, NPOS], BF, name=f"virt{t}") for t in range(4)]
            rows_cm = tc.tile_pool(name="rows", bufs=2)
            rows = rows_cm.__enter__()
            ones64 = pp.tile([CAP, 1], BF, name="ones64")
            nc.vector.memset(ones64[:], 1.0)
            ones1b = pp.tile([1, 128], BF, name="ones1b")
            nc.vector.memset(ones1b[:], 1.0)
            onesf = pp.tile([128, 1], BF, name="onesf")
            nc.vector.memset(onesf[:], 1.0)

            vstp_cm = tc.tile_pool(name="vstp", bufs=2)
            vstp = vstp_cm.__enter__()
            qkp_cm = tc.tile_pool(name="qk", bufs=1)
            qkp = qkp_cm.__enter__()
            q_s = [qkp.tile([128, NPOS], BF, name=f"q{t}") for t in range(4)]
            k_s = [qkp.tile([128, NPOS], BF, name=f"k{t}") for t in range(4)]

            # ---------------- phase 1: q,k convs ----------------
            ndrain = 0
            for wd, dst in ((wq_d, q_s), (wk_d, k_s)):
                for cto in range(4):
                    wt = wts.tile([128, 4, 9, 128], BF, name="wt", tag="wt")
                    for ci in range(4):
                        nc.sync.dma_start(out=wt[:, ci, :, :], in_=wslice_ap(wd, ci, cto))
                    for b in range(7):
                        acc = pconv.tile([128, W7], F32, name="acc", tag="acc")
                        mms = conv_row_mms(b)
                        for mi, (ci, ti, so, do, rx) in enumerate(mms):
                            rhs = bass.AP(tensor=xt[ci].tensor,
                                          offset=xt[ci].offset + so,
                                          ap=[xt[ci].ap[0], [CAP, rx], [1, CAP]])
                            outap = bass.AP(tensor=acc.tensor, offset=acc.offset + do,
                                            ap=[acc.ap[0], [CAP, rx], [1, CAP]])
                            nc.tensor.matmul(outap, wt[:, ci, ti, :], rhs,
                                             start=(mi == 0), stop=(mi == len(mms) - 1),
                                             skip_group_check=True)
                        dslice = dst[cto][:, b * W7:(b + 1) * W7]
                        if ndrain % 2 == 0:
                            nc.vector.tensor_copy(dslice, acc[:])
                        else:
                            nc.scalar.activation(dslice, acc[:], func=AF.Copy)
                        ndrain += 1

            # ---------------- phase 2a: QK^T + mask + exp + rowsum ----------------
            rsum = rows.tile([1, NPOS], F32, name="rsum", tag="row")
            rec_bf = pp.tile([1, NPOS], BF, name="recbf")
            for pg in range(7):
                aps = pqk.tile([CAP, W7], F32, name="aps", tag="aps")
                for pp_i in range(7):
                    p = pg * 7 + pp_i
                    for ct in range(4):
                        lhsT = k_s[ct][:, p * CAP:(p + 1) * CAP]
                        rhs = q_s[ct][:, p * CAP:(p + 1) * CAP]
                        nc.tensor.matmul(aps[:, pp_i * CAP:(pp_i + 1) * CAP],
                                         lhsT, rhs, start=(ct == 0), stop=(ct == 3))
                attf = drp.tile([CAP, W7], F32, name="attf", tag="attf")
                nc.vector.tensor_add(attf[:], aps[:], mask7[:])
                nc.scalar.activation(attw[:, pg * W7:(pg + 1) * W7], attf[:],
                                     func=AF.Exp)
                rs_ps = pst.tile([1, W7], F32, name="rsps", tag="rsps")
                nc.tensor.matmul(rs_ps[:], ones64[:],
                                 attw[:, pg * W7:(pg + 1) * W7],
                                 start=True, stop=True)
                nc.vector.tensor_copy(rsum[:, pg * W7:(pg + 1) * W7], rs_ps[:])
            nc.vector.reciprocal(rsum[:], rsum[:])
            nc.vector.tensor_copy(rec_bf[:], rsum[:])
            qkp_cm.__exit__(None, None, None)
            vthp_cm = tc.tile_pool(name="vthp", bufs=2)
            vthp = vthp_cm.__enter__()
            rpp_cm = tc.tile_pool(name="rpp", bufs=1)
            rpp = rpp_cm.__enter__()
            rp = [rpp.tile([128, NPOS], BF, name=f"rp{t}") for t in range(4)]

            # ---------------- v conv ----------------
            def v_conv_cto(cto):
                wt = wts.tile([128, 4, 9, 128], BF, name="wtv", tag="wt")
                for ci in range(4):
                    nc.sync.dma_start(out=wt[:, ci, :, :], in_=wslice_ap(wv_d, ci, cto))
                vs_t = vstp.tile([128, NPOS], BF, name="vst", tag="vst")
                for b in range(7):
                    acc = pconv.tile([128, W7], F32, name="accv", tag="acc")
                    mms = conv_row_mms(b)
                    for mi, (ci, ti, so, do, rx) in enumerate(mms):
                        rhs = bass.AP(tensor=xt[ci].tensor, offset=xt[ci].offset + so,
                                      ap=[xt[ci].ap[0], [CAP, rx], [1, CAP]])
                        outap = bass.AP(tensor=acc.tensor, offset=acc.offset + do,
                                        ap=[acc.ap[0], [CAP, rx], [1, CAP]])
                        nc.tensor.matmul(outap, wt[:, ci, ti, :], rhs,
                                         start=(mi == 0), stop=(mi == len(mms) - 1),
                                         skip_group_check=True)
                    # transpose-drain (pp,i) -> (i,p): DVE strides are free
                    sap = bass.AP(tensor=acc.tensor, offset=acc.offset,
                                  ap=[acc.ap[0], [1, CAP], [CAP, 7]])
                    dap = bass.AP(tensor=vs_t.tensor, offset=vs_t.offset + b * 7,
                                  ap=[vs_t.ap[0], [P, CAP], [1, 7]])
                    nc.vector.tensor_copy(dap, sap)
                # one store: dst per-partition runs of P elems (98B)
                dstap = bass.AP(tensor=v_dram[:].tensor, offset=cto * 128 * P,
                                ap=[[P, 128], [4 * 128 * P, CAP], [1, P]])
                sap = bass.AP(tensor=vs_t.tensor, offset=vs_t.offset,
                              ap=[vs_t.ap[0], [P, CAP], [1, P]])
                nc.sync.dma_start(out=dstap, in_=sap)

            v_conv_cto(0)
            v_conv_cto(1)

            # recip bcast + attw scale (tensor MMs land after v cto1 in engine order)
            for pg in range(7):
                rb = pav.tile([CAP, W7], F32, name="rb", tag="rb")
                rhs = bass.AP(tensor=rec_bf.tensor, offset=rec_bf.offset + pg * W7,
                              ap=[rec_bf.ap[0], [1, W7]])
                o1 = bass.AP(tensor=ones1b.tensor, offset=ones1b.offset,
                             ap=[ones1b.ap[0], [1, CAP]])
                nc.tensor.matmul(rb[:], o1, rhs, start=True, stop=True)
                aw = attw[:, pg * W7:(pg + 1) * W7]
                nc.vector.tensor_mul(aw, aw, rb[:])

            v_conv_cto(2)

            # ---------------- AV ----------------
            def av_ct(ct):
                vth = vthp.tile([CAP, 128, P], BF, name=f"vth{ct}", tag="vth")
                src = bass.AP(tensor=v_dram[:].tensor, offset=ct * 128 * P,
                              ap=[[4 * 128 * P, CAP], [1, 128 * P]])
                nc.sync.dma_start(out=vth[:], in_=src)
                for pg in range(7):
                    av = pav.tile([128, W7], F32, name="av", tag="rb")
                    for pp_i in range(7):
                        p = pg * 7 + pp_i
                        lhsT = bass.AP(tensor=vth.tensor, offset=vth.offset + p,
                                       ap=[vth.ap[0], [P, 128]])
                        nc.tensor.matmul(av[:, pp_i * CAP:(pp_i + 1) * CAP],
                                         lhsT,
                                         attw[:, p * CAP:(p + 1) * CAP],
                                         start=True, stop=True)
                    dslice = virt[ct][:, pg * W7:(pg + 1) * W7]
                    if (ct + pg) % 2 == 0:
                        nc.vector.tensor_copy(dslice, av[:])
                    else:
                        nc.scalar.activation(dslice, av[:], func=AF.Copy)

            av_ct(0)
            av_ct(1)
            v_conv_cto(3)
            av_ct(2)
            av_ct(3)

            # ---------------- GN stats ----------------
            s1 = rows.tile([1, NPOS], F32, name="s1", tag="row")
            s2 = rows.tile([1, NPOS], F32, name="s2", tag="row")
            for pg in range(7):
                sqs = []
                for ct in range(4):
                    sq = drp.tile([128, W7], BF, name="sq", tag="attf")
                    nc.scalar.activation(sq[:], virt[ct][:, pg * W7:(pg + 1) * W7],
                                         func=AF.Square)
                    sqs.append(sq)
                s1_ps = pst.tile([1, W7], F32, name="s1ps", tag="rsps")
                s2_ps = pst.tile([1, W7], F32, name="s2ps", tag="rsps")
                for ct in range(4):
                    nc.tensor.matmul(s1_ps[:], onesf[:],
                                     virt[ct][:, pg * W7:(pg + 1) * W7],
                                     start=(ct == 0), stop=(ct == 3))
                for ct in range(4):
                    nc.tensor.matmul(s2_ps[:], onesf[:], sqs[ct][:],
                                     start=(ct == 0), stop=(ct == 3))
                nc.vector.tensor_copy(s1[:, pg * W7:(pg + 1) * W7], s1_ps[:])
                nc.vector.tensor_copy(s2[:, pg * W7:(pg + 1) * W7], s2_ps[:])

            s1i = pp.tile([1, CAP], F32, name="s1i")
            s2i = pp.tile([1, CAP], F32, name="s2i")
            for src_t, dsti in ((s1, s1i), (s2, s2i)):
                v3 = bass.AP(tensor=src_t.tensor, offset=src_t.offset,
                             ap=[src_t.ap[0], [1, CAP], [CAP, P]])
                nc.vector.reduce_sum(dsti[:], v3, axis=mybir.AxisListType.X)
            inv_n = 1.0 / (C * P)
            mean_r = pp.tile([1, CAP], F32, name="meanr")
            var_r = pp.tile([1, CAP], F32, name="varr")
            msq = pp.tile([1, CAP], F32, name="msq")
            eps_t = pp.tile([1, 1], F32, name="eps")
            nc.vector.memset(eps_t[:], 1e-5)
            nc.vector.tensor_scalar_mul(mean_r[:], s1i[:], inv_n)
            nc.vector.tensor_scalar_mul(var_r[:], s2i[:], inv_n)
            nc.vector.tensor_mul(msq[:], mean_r[:], mean_r[:])
            nc.vector.tensor_sub(var_r[:], var_r[:], msq[:])
            rstd_bf = pp.tile([1, CAP], BF, name="rstdbf")
            negb_bf = pp.tile([1, CAP], BF, name="negbbf")
            nc.scalar.activation(var_r[:], var_r[:], func=AF.Sqrt,
                                 bias=eps_t[:], scale=1.0)
            nc.vector.reciprocal(var_r[:], var_r[:])   # var_r := rstd
            nc.vector.tensor_mul(msq[:], mean_r[:], var_r[:])
            nc.vector.tensor_scalar_mul(msq[:], msq[:], -1.0)  # msq := negb
            nc.vector.tensor_copy(rstd_bf[:], var_r[:])
            nc.vector.tensor_copy(negb_bf[:], msq[:])

            # ---------------- normalize + relu (per pg, pipelined) ----------------
            for pg in range(7):
                rstd_ps = pav.tile([128, W7], F32, name="rstdps", tag="rb")
                negb_ps = pst.tile([128, W7], F32, name="negbps", tag="rsps")
                rr = bass.AP(tensor=rstd_bf.tensor, offset=rstd_bf.offset,
                             ap=[rstd_bf.ap[0], [0, 7], [1, CAP]])
                nb = bass.AP(tensor=negb_bf.tensor, offset=negb_bf.offset,
                             ap=[negb_bf.ap[0], [0, 7], [1, CAP]])
                nc.tensor.matmul(rstd_ps[:], ones1b[:], rr, start=True, stop=True)
                nc.tensor.matmul(negb_ps[:], ones1b[:], nb, start=True, stop=True)
                for ct in range(4):
                    t = drp.tile([128, W7], BF, name="tno", tag="attf")
                    vslice = virt[ct][:, pg * W7:(pg + 1) * W7]
                    nc.vector.tensor_mul(t[:], vslice, rstd_ps[:])
                    nc.vector.tensor_add(t[:], t[:], negb_ps[:])
                    nc.scalar.activation(rp[ct][:, pg * W7:(pg + 1) * W7], t[:],
                                         func=AF.Relu)

            # ---------------- phase 3: out conv + residual ----------------
            for cto in range(4):
                wt = wts.tile([128, 4, 9, 128], BF, name="wt3", tag="wt")
                for ci in range(4):
                    nc.sync.dma_start(out=wt[:, ci, :, :], in_=wslice_ap(wo_d, ci, cto))
                for b in range(7):
                    xit = drp.tile([128, W7], F32, name="xit", tag="xit")
                    nc.sync.dma_start(
                        out=xit[:], in_=xint_d[cto][:, b * W7:(b + 1) * W7])
                    acc = pconv.tile([128, W7], F32, name="acc3", tag="acc")
                    mms = conv_row_mms(b)
                    for mi, (ci, ti, so, do, rx) in enumerate(mms):
                        rhs = bass.AP(tensor=rp[ci].tensor, offset=rp[ci].offset + so,
                                      ap=[rp[ci].ap[0], [CAP, rx], [1, CAP]])
                        outap = bass.AP(tensor=acc.tensor, offset=acc.offset + do,
                                        ap=[acc.ap[0], [CAP, rx], [1, CAP]])
                        nc.tensor.matmul(outap, wt[:, ci, ti, :], rhs,
                                         start=(mi == 0), stop=(mi == len(mms) - 1),
                                         skip_group_check=True)
                    o = drp.tile([128, W7], F32, name="o", tag="o")
                    nc.vector.tensor_add(o[:], acc[:], xit[:])
                    nc.sync.dma_start(out=y_d[cto][:, b * W7:(b + 1) * W7], in_=o[:])

            rpp_cm.__exit__(None, None, None)
            vthp_cm.__exit__(None, None, None)
            vstp_cm.__exit__(None, None, None)
            rows_cm.__exit__(None, None, None)

    _split_multiwait(nc)
    _NC_CACHE[CAP] = nc
    return nc


def _shard(rois):
    """Pack videos onto 8 cores: exact-64 partition if possible, else LPT."""
    vid = rois[:, 0].astype(np.int64)
    sizes = np.bincount(vid, minlength=vid.max() + 1)
    nvid = len(sizes)
    total = int(sizes.sum())
    target = total // NCORE
    v2c = None
    if total % NCORE == 0:
        import random
        rng = random.Random(0)
        for _ in range(300):
            remaining = set(range(nvid))
            bins = []
            ok = True
            for _b in range(NCORE):
                items = [v for v in remaining]
                rng.shuffle(items)
                reach = {0: []}
                for v in items:
                    s = int(sizes[v])
                    if s == 0:
                        continue
                    new = {}
                    for tot, sub in reach.items():
                        t2 = tot + s
                        if t2 <= target and t2 not in reach and t2 not in new:
                            new[t2] = sub + [v]
                    reach.update(new)
                    if target in reach:
                        break
                if target not in reach:
                    ok = False
                    break
                sub = reach[target]
                bins.append(sub)
                remaining -= set(sub)
            if ok and not any(sizes[v] > 0 for v in remaining):
                v2c = np.zeros(nvid, np.int64)
                for c, b in enumerate(bins):
                    for v in b:
                        v2c[v] = c
                break
    if v2c is None:
        order = np.argsort(-sizes, kind='stable')
        loads = np.zeros(NCORE, np.int64)
        v2c = np.zeros(nvid, np.int64)
        for v in order:
            c = int(np.argmin(loads))
            loads[c] += sizes[v]
            v2c[v] = c
    core_of_roi = v2c[vid]
    idxs = [np.nonzero(core_of_roi == c)[0] for c in range(NCORE)]
    cap = max(64, max(len(ix) for ix in idxs))
    assert 7 * cap * 4 <= 2048 * 1, f"row bank overflow: cap={cap}"
    return idxs, vid, cap


def kernel(x, rois, w_q, w_k, w_v, w_out, gamma, beta):
    _install_profhook()
    x = np.asarray(x, np.float32)
    rois = np.asarray(rois)
    assert np.allclose(np.asarray(gamma), 1.0) and np.allclose(np.asarray(beta), 0.0), \
        "kernel folds GN affine assuming gamma=1, beta=0"
    idxs, vid, CAP = _shard(rois)
    nc = _build(CAP)
    NPOS = CAP * P

    def wprep(w, scale=1.0):
        # [co, ci, 1, 3, 3] -> [ci(4,128), tap(ky*3+kx), co(4,128)] bf16
        a = (np.asarray(w, np.float32)[:, :, 0] * scale).transpose(1, 2, 3, 0)
        return np.ascontiguousarray(
            a.reshape(4, 128, 9, 4, 128)).astype(ml_dtypes.bfloat16)

    wq = wprep(w_q, 1.0 / np.sqrt(np.float32(C)))
    wk, wv, wo = wprep(w_k), wprep(w_v), wprep(w_out)

    in_maps = []
    for c in range(NCORE):
        ix = idxs[c]
        n = len(ix)
        # (p,i)-major: [C, 49, CAP]
        xq = np.zeros((C, P, CAP), np.float32)
        xq[:, :, :n] = x[ix, :, 0].reshape(n, C, P).transpose(1, 2, 0)
        xp = np.ascontiguousarray(xq.reshape(4, 128, NPOS)).astype(ml_dtypes.bfloat16)
        xint = np.ascontiguousarray(xq.reshape(4, 128, NPOS))
        ids = np.full(CAP, -1, np.int64)
        ids[:n] = vid[ix]
        ids[n:] = 10 ** 6 + np.arange(CAP - n)
        mask = np.where(ids[:, None] == ids[None, :], 0.0, -1e30).astype(np.float32)
        in_maps.append(dict(xp=xp, xint=xint, wq=wq, wk=wk, wv=wv, wo=wo, mask=mask))

    res = run_bass_kernel_spmd(nc, in_maps, list(range(NCORE)))
    kernel.last_exec_ns = res.exec_time_ns

    out = np.empty((512, C, 1, 7, 7), np.float32)
    for c in range(NCORE):
        ix = idxs[c]
        n = len(ix)
        yc = res.results[c]["y"].reshape(C, P, CAP).transpose(2, 0, 1)
        out[ix] = yc[:n].reshape(n, C, 1, 7, 7)
    return out
